# revision 20
# baseline (speedup 1.0000x reference)
"""ClassBalancedSupConLoss on 8 TRN2 NeuronCores (Bass/Tile), v3.

Sharding: the BANK is column-sharded across the 8 cores (2048 cols each,
class-balanced with uniform cut positions), every core holds the full
(class-sorted) batch as matmul columns plus its own 256 anchors as
stationary weights. Each core computes, for ALL 2048 anchors, exp-sums
against its bank slice, and for its OWN anchors the batch (bb) exp-sum
total, self term, and positives row-sums. The host (numpy, fp64)
assembles denominators, logs, and the masked mean.

Work split per [128, 2048] chunk (18 per core):
  PE : 4 x [128, 512] matmuls into a rotating PSUM tile
  ACT: one Exp pass with accum_out = chunk total (free during ACTIVATE)
  DVE: own-class-range segment reduce(s) of the bf16 exp output
Pure anchor tiles (all 128 sorted anchors share a class -- 14 of 16)
need only ONE fixed-range DVE reduce: denominator = total - own-class.
The <= 2 class-straddling tiles fall back to 3 per-segment reduces.
bb chunks need NO reduce at all: denominator_bb = total - selfe, with
selfe = exp(inv_t*(s_ii-1)) computed from the same rounded operands so
the ~1.0 self term cancels at ACT-internal precision.

Class balancing: per class c every core gets exactly q_c =
floor(mcnt_c/8) bank columns; the per-core shortfall (2048 - sum q_c)
is zero-vector dummy columns whose exact exp(-inv_t) contribution the
host subtracts, and the <= 21 leftover real columns are folded in on
the host. Cut positions q0, q0+q1 are therefore compile-time constants
shared by all cores (SPMD-safe fixed-range reduces).

Numerics: matmul inputs fp8 e4m3 (logit noise washes out over the
18k-term sums; positives/self are consistent because host corrections
reuse the same quantized operands); exp outputs bf16; device sums fp32;
host assembly fp64.
"""

import os
import numpy as np

import concourse.bass as bass  # noqa: F401
from concourse import bacc
import concourse.mybir as mybir
import concourse.tile as tile
from concourse.bass_utils import run_bass_kernel_spmd

B, D, M, C = 2048, 128, 16384, 3
NCORES = 8
APC = B // NCORES          # own anchors per core = 256
NT = B // 128              # anchor tiles = 16 (all anchors)
NOWN = APC // 128          # own anchor tiles = 2
CH = 512                   # matmul free chunk (one PSUM bank)
W = 2048                   # chunk width = one PSUM [128, 2048] tile
BASE_TEMP = 0.07

F32 = mybir.dt.float32
BF16 = mybir.dt.bfloat16
AF = mybir.ActivationFunctionType
ALU = mybir.AluOpType
AX = mybir.AxisListType

# "f8"  : fp8 e4m3 matmul inputs (default)
# "bf16": bfloat16 matmul inputs (2x DMA bytes, less logit noise)
MM_MODE = os.environ.get("SUPCON_MM_MODE", "f8")
WARMUP = int(os.environ.get("SUPCON_WARMUP", "0"))
# number of pure bank chunks whose exp runs on the DVE (Schraudolph
# int16/bf16 bit-trick) instead of the saturated ACT engine
NOFF = int(os.environ.get("SUPCON_OFFLOAD", "0"))

# bf16-domain Schraudolph exp: bitcast(int16(A16*y + B16)) ~= e^y
A16 = 128.0 / np.log(2.0)
B16_BASE = 127.0 * 128.0

LAST_EXEC_TIME_NS = None   # set by kernel() when SUPCON_TRACE=1

# oout column layout (per core, [128, OC] fp32)
OC_SEG = 0                 # 16 tiles x 3: pure = (T, own, -) / straddle = (s0, s1, s2)
OC_BBT = 48                # 2 own tiles: bb totals
OC_SELFE = 50              # 2
OC_RAW3 = 52               # 2 own tiles x 3 = 6
OC_SDIAG = 58              # 2
OC = 60


def _install_trace_shim():
    """Register the NTFF profile hook that this image's antenv lacks."""
    import sys
    import types
    import ctypes
    import contextlib

    try:
        from antenv.axon_hooks import get_axon_ntff_profile_hook  # noqa: F401
        return True  # real module exists
    except ImportError:
        pass

    so_path = "/opt/axon/libaxon_pjrt.so"
    if not os.path.exists(so_path):
        return False
    lib = ctypes.CDLL(so_path)
    if not hasattr(lib, "axon_start_nrt_profile"):
        return False
    lib.axon_start_nrt_profile.argtypes = [
        ctypes.POINTER(ctypes.c_int64),
        ctypes.c_size_t,
    ]
    lib.axon_start_nrt_profile.restype = ctypes.c_int64
    lib.axon_stop_nrt_profile.argtypes = [ctypes.c_char_p]
    lib.axon_stop_nrt_profile.restype = ctypes.c_int64

    @contextlib.contextmanager
    def _hook(output_dir, device_ids):
        import jax

        jax.devices()
        if device_ids:
            ids = (ctypes.c_int64 * len(device_ids))(*device_ids)
            rc = lib.axon_start_nrt_profile(ids, len(device_ids))
        else:
            rc = lib.axon_start_nrt_profile(None, 0)
        if rc != 0:
            raise RuntimeError(f"axon_start_nrt_profile rc={rc}")
        try:
            yield
        finally:
            n = lib.axon_stop_nrt_profile(str(output_dir).encode())
            print(f"profile: {n} file(s) written to {output_dir}", file=sys.stderr)

    _state = {"hook": _hook}
    mod = types.ModuleType("antenv.axon_hooks")
    mod.get_axon_ntff_profile_hook = lambda: _state["hook"]
    mod.set_axon_ntff_profile_hook = lambda h: _state.update(hook=h)
    sys.modules["antenv.axon_hooks"] = mod
    import antenv

    antenv.axon_hooks = mod

    import concourse.bass_utils as bu

    bu.upload_artifacts = lambda tmpdir: tmpdir
    return True


def _build(c1, c2, tile_cls, off_tiles, mm_mode):
    """c1/c2: class cuts in every core's bank slice; tile_cls[t]: class of
    anchor tile t if pure, else None (straddles a batch class boundary);
    off_tiles: pure bank tiles whose exp runs on the DVE (bit-trick)."""
    import ml_dtypes  # noqa: F401

    in_dt = mybir.dt.float8e4 if mm_mode == "f8" else BF16
    I16 = mybir.dt.int16

    AW = APC + 8               # anchT width: 256 own + 3 gT + 5 pad
    # invt | ninvt | invt_own | ninvt_own | sA | sB | eye
    NV = 4 * NT + 2 * NOWN + 128

    nc = bacc.Bacc()
    embT_d = nc.declare_dram_parameter("embT", [D, W], in_dt, isOutput=False)
    anchT_d = nc.declare_dram_parameter("anchT", [D, AW], in_dt, isOutput=False)
    bankT_d = nc.declare_dram_parameter("bankT", [D, W], in_dt, isOutput=False)
    vecs_d = nc.declare_dram_parameter("vecs", [128, NV], F32, isOutput=False)
    oout_d = nc.declare_dram_parameter("oout", [128, OC], F32, isOutput=True)

    seg_r = [(0, c1), (c1, c2), (c2, W)]

    with tile.TileContext(nc) as tc:
        with (
            tc.tile_pool(name="big", bufs=1) as bigp,
            tc.tile_pool(name="sm", bufs=1) as smp,
            tc.tile_pool(name="scr", bufs=2) as scrp,
            tc.tile_pool(name="i16", bufs=2) as i16p,
            tc.tile_pool(name="ps", bufs=2, space="PSUM") as psp,
        ):
            emb_t = bigp.tile([D, W], in_dt, tag="embT")
            anch_t = bigp.tile([D, AW], in_dt, tag="anchT")
            bank_t = bigp.tile([D, W], in_dt, tag="bankT")
            vecs_t = smp.tile([128, NV], F32, tag="vecs")
            o = [0]

            def vslice(w):
                a = o[0]; o[0] += w
                return vecs_t[:, a:a + w]
            invt_t = vslice(NT)
            ninvt_t = vslice(NT)
            invo_t = vslice(NOWN)
            ninvo_t = vslice(NOWN)
            sA_t = vslice(NT)
            sB_t = vslice(NT)
            eye_t = vslice(128)
            # garbage-operand warmup tiles (never written)
            junkw_t = bigp.tile([128, 128], in_dt, tag="junkw")
            junkx_t = bigp.tile([128, CH], in_dt, tag="junkx")

            oout_t = smp.tile([128, OC], F32, tag="oout")
            eyemul = smp.tile([128, 128], F32, tag="eyemul")
            warm = smp.tile([128, 1], F32, tag="warm")
            scr2 = smp.tile([128, W], BF16, tag="scr2")
            sdiag = [smp.tile([128, 1], F32, tag=f"sdiag{t}", name=f"sdiag{t}")
                     for t in range(NOWN)]

            # ACT first: exp table load + warm activation on junk data, so
            # the ~2.7us table load runs during the input DMA window.
            nc.vector.memset(junkx_t[:, 0:1], 0.0)
            nc.scalar.activation(warm[:], junkx_t[:, 0:1], AF.Exp)

            # input DMA: sync queue carries vecs + emb + anchT (needed
            # first), scalar queue carries the bank slice (needed later).
            H = W // 2
            nc.sync.dma_start(out=vecs_t[:], in_=vecs_d[:])
            nc.sync.dma_start(out=anch_t[:], in_=anchT_d[:])
            nc.sync.dma_start(out=emb_t[:, 0:H], in_=embT_d[:, 0:H])
            nc.sync.dma_start(out=emb_t[:, H:W], in_=embT_d[:, H:W])
            nc.scalar.dma_start(out=bank_t[:], in_=bankT_d[:])

            # PE warmup on garbage operands (HAM clock-gate opener)
            if WARMUP:
                nc.vector.memset(junkw_t[:], 0.0)
                nc.vector.memset(junkx_t[:], 0.0)
                warm_ps = psp.tile([128, W], F32, tag="chunk", name="warm_ps")
                for w in range(WARMUP):
                    nc.tensor.matmul(
                        warm_ps[:, (w % 4) * CH:((w % 4) + 1) * CH],
                        junkw_t[:], junkx_t[:], start=True, stop=True,
                    )

            def ts_sum(src, width, col):
                """oout[col] = sum(src) on the DVE (TENSOR_REDUCE, 1x)."""
                nc.vector.reduce_sum(oout_t[:, col:col + 1], src, axis=AX.X)

            def emit_chunk(lhs, moving, sc, bi, accum, reduces, off=None):
                """[128, 2048] chunk: 4 matmuls + exp + range sums.

                ACT path: Exp (scale sc / bias bi) with optional accum col,
                then DVE range sums of the bf16 output.
                DVE path (off=(sa, sb)): Schraudolph int16 convert + bitcast
                bf16 range sums; ACT untouched."""
                ps = psp.tile([128, W], F32, tag="chunk", name="ps")
                for q in range(W // CH):
                    nc.tensor.matmul(
                        ps[:, q * CH:(q + 1) * CH], lhs,
                        moving[:, q * CH:(q + 1) * CH],
                        start=True, stop=True,
                    )
                if off is not None:
                    sa, sb = off
                    i16 = i16p.tile([128, W], I16, tag="i16", name="i16")
                    nc.vector.tensor_scalar(
                        out=i16[:], in0=ps[:], scalar1=sa, scalar2=sb,
                        op0=ALU.mult, op1=ALU.add)
                    for (a, b, col) in reduces:
                        ts_sum(i16[:, a:b].bitcast(BF16), b - a, col)
                else:
                    kw = {}
                    if accum is not None:
                        kw["accum_out"] = oout_t[:, accum:accum + 1]
                    # exp in place in PSUM: ScE sits closer to PSUM and no
                    # SBUF write contends with the DVE's segment reads
                    nc.scalar.activation(
                        ps[:], ps[:], AF.Exp, bias=bi, scale=sc, **kw)
                    for (a, b, col) in reduces:
                        ts_sum(ps[:, a:b], b - a, col)

            def own(t):
                return anch_t[:, t * 128:(t + 1) * 128]

            # bb chunks for the 2 own tiles: total only (host does T - selfe)
            for t in range(NOWN):
                emit_chunk(own(t), emb_t,
                           invo_t[:, t:t + 1], ninvo_t[:, t:t + 1],
                           OC_BBT + t, [])

            # bank chunks for all 16 anchor tiles.  Pure tiles: ACT accum
            # gives the chunk total, one DVE reduce gives the own-class
            # segment (host: den += T - own).
            for t in range(NT):
                lhs = emb_t[:, t * 128:(t + 1) * 128]
                sc, bi = invt_t[:, t:t + 1], ninvt_t[:, t:t + 1]
                if tile_cls[t] is not None:
                    a, bnd = seg_r[tile_cls[t]]
                    emit_chunk(lhs, bank_t, sc, bi, OC_SEG + t * 3,
                               [(a, bnd, OC_SEG + t * 3 + 1)])
                else:
                    rd = [(a, bnd, OC_SEG + t * 3 + ci)
                          for ci, (a, bnd) in enumerate(seg_r) if bnd > a]
                    emit_chunk(lhs, bank_t, sc, bi, None, rd)

            # epilogue: self-similarity diag + positives row-sums
            post_ps = psp.tile([128, W], F32, tag="chunk", name="post_ps")
            for t in range(NOWN):
                nc.tensor.matmul(
                    post_ps[:, t * 128:(t + 1) * 128], own(t), own(t),
                    start=True, stop=True,
                )
            for t in range(NOWN):
                nc.tensor.matmul(
                    post_ps[:, 256 + t * C:256 + (t + 1) * C], own(t),
                    anch_t[:, APC:APC + C], start=True, stop=True,
                )
            for t in range(NOWN):
                nc.vector.tensor_mul(
                    eyemul[:], post_ps[:, t * 128:(t + 1) * 128], eye_t[:])
                nc.vector.reduce_sum(sdiag[t][:], eyemul[:], axis=AX.X)
                nc.scalar.activation(
                    oout_t[:, OC_SELFE + t:OC_SELFE + t + 1], sdiag[t][:],
                    AF.Exp, bias=ninvo_t[:, t:t + 1], scale=invo_t[:, t:t + 1],
                )
                nc.vector.tensor_copy(
                    out=oout_t[:, OC_SDIAG + t:OC_SDIAG + t + 1], in_=sdiag[t][:])
            nc.vector.tensor_copy(
                out=oout_t[:, OC_RAW3:OC_RAW3 + NOWN * C],
                in_=post_ps[:, 256:256 + NOWN * C])

            nc.sync.dma_start(out=oout_d[:], in_=oout_t[:])

    nc.compile()
    return nc


def kernel(embeddings, labels, bank_embs, bank_labels, class_temps):
    global LAST_EXEC_TIME_NS
    import ml_dtypes

    f8 = ml_dtypes.float8_e4m3
    in_np = f8 if MM_MODE == "f8" else ml_dtypes.bfloat16

    emb = np.asarray(embeddings, dtype=np.float32)
    bank = np.asarray(bank_embs, dtype=np.float32)
    lab = np.asarray(labels).astype(np.int64).ravel()
    blab = np.asarray(bank_labels).astype(np.int64).ravel()
    ct = np.asarray(class_temps, dtype=np.float32).ravel()

    # sort batch and bank by class
    bord = np.argsort(lab, kind="stable")
    slab = lab[bord]
    emb_s = emb[bord]                                  # [B, D] f32, sorted
    cnt = np.bincount(lab, minlength=C)
    mord = np.argsort(blab, kind="stable")
    bank_s = bank[mord]
    mcnt = np.bincount(blab, minlength=C)

    # per-core class quotas (even, for 4B-aligned bf16 reduce ranges)
    q = ((mcnt // NCORES) // 2 * 2).astype(np.int64)   # [3]
    sdum = int(W - q.sum())                            # zero-dummy cols/core
    assert sdum >= 0
    c1, c2 = int(q[0]), int(q[0] + q[1])
    cls_off = np.concatenate([[0], np.cumsum(mcnt)[:-1]])

    # anchor-tile purity (compile-time, same for all cores)
    tile_cls = []
    for t in range(NT):
        c_lo, c_hi = slab[t * 128], slab[t * 128 + 127]
        tile_cls.append(int(c_lo) if c_lo == c_hi else None)

    # quantized operands (shared by device and host-side corrections)
    embq = emb_s.astype(in_np)                         # [B, D]
    bankq = bank_s.astype(in_np)
    embq_f = embq.astype(np.float32)
    bankq_f = bankq.astype(np.float32)
    g = np.stack([emb_s[slab == c].sum(axis=0) for c in range(C)], axis=1)
    gq = g.astype(in_np)                               # [D, 3]

    inv_t_all = (1.0 / ct[slab]).astype(np.float32)    # [B] per sorted anchor

    # DVE-offloaded pure bank tiles, spread across the chunk sequence
    pure = [t for t in range(NT) if tile_cls[t] is not None]
    noff = min(NOFF, len(pure))
    off_tiles = (set(pure[int(i)] for i in
                     np.linspace(0, len(pure) - 1, noff).round())
                 if noff > 0 else set())

    # per-class Schraudolph bias tuning: pick corr_c that zeroes the mean
    # relative error of bitcast(int16(A16*y + B16 - corr)) over the y
    # distribution of this class's logits (s ~ N(0, 1/sqrt(D)))
    corr_cls = np.zeros(C)
    if off_tiles:
        sgrid = np.linspace(-4.0, 4.0, 4001) / np.sqrt(D)
        wpdf = np.exp(-0.5 * (sgrid * np.sqrt(D)) ** 2)
        for c in range(C):
            it = 1.0 / float(ct[c])
            y = it * (sgrid - 1.0)
            exact = np.exp(y)
            wexp = wpdf * exact
            best, bestv = 0.0, np.inf
            for corr in np.linspace(0.0, 12.0, 121):
                i16v = np.clip(np.rint(A16 * y + B16_BASE - corr), 0, 32767)
                approx = i16v.astype(np.int16).view(ml_dtypes.bfloat16).astype(np.float64)
                bias = abs(np.sum(wpdf * approx) / np.sum(wpdf * exact) - 1.0)
                if bias < bestv:
                    best, bestv = corr, bias
            corr_cls[c] = best

    nc = _build(c1, c2, tile_cls, off_tiles, MM_MODE)

    eye128 = np.eye(128, dtype=np.float32)
    embT = np.ascontiguousarray(embq.T)                # [D, B], shared
    invt_cols = np.ascontiguousarray(inv_t_all.reshape(NT, 128).T)
    sA_cols = (A16 * invt_cols).astype(np.float32)
    corr_all = corr_cls[slab]
    sB_all = (B16_BASE - corr_all - A16 * inv_t_all.astype(np.float64))
    sB_cols = np.ascontiguousarray(sB_all.reshape(NT, 128).T).astype(np.float32)
    in_maps = []
    for k in range(NCORES):
        asl = slice(k * APC, (k + 1) * APC)
        anchT = np.zeros((D, APC + 8), dtype=in_np)
        anchT[:, 0:APC] = embq[asl].T
        anchT[:, APC:APC + C] = gq
        bankT = np.zeros((D, W), dtype=in_np)
        pos = 0
        for c in range(C):
            sel = bankq[cls_off[c] + k * q[c]: cls_off[c] + (k + 1) * q[c]]
            bankT[:, pos:pos + q[c]] = sel.T
            pos += int(q[c])
        ivo = inv_t_all[asl]
        vecs = np.concatenate([
            invt_cols, -invt_cols,
            np.ascontiguousarray(ivo.reshape(NOWN, 128).T),
            np.ascontiguousarray((-ivo).reshape(NOWN, 128).T),
            sA_cols, sB_cols,
            eye128,
        ], axis=1).astype(np.float32)
        in_maps.append({
            "embT": embT,
            "anchT": np.ascontiguousarray(anchT),
            "bankT": np.ascontiguousarray(bankT),
            "vecs": np.ascontiguousarray(vecs),
        })

    trace = os.environ.get("SUPCON_TRACE", "0") == "1"
    if trace:
        trace = _install_trace_shim()
    res = run_bass_kernel_spmd(nc, in_maps, core_ids=list(range(NCORES)), trace=trace)
    LAST_EXEC_TIME_NS = res.exec_time_ns

    # ---- host assembly (fp64) ----
    inv64 = inv_t_all.astype(np.float64)
    den = np.zeros(B, dtype=np.float64)
    raw3_own = np.zeros(B, dtype=np.float64)
    sdiag_own = np.zeros(B, dtype=np.float64)
    tidx = np.arange(128)
    for k in range(NCORES):
        oo = np.asarray(res.results[k]["oout"], dtype=np.float64)  # [128, OC]
        for t in range(NT):
            a_idx = t * 128 + tidx
            if tile_cls[t] is not None:
                T = oo[:, OC_SEG + t * 3]
                own_s = oo[:, OC_SEG + t * 3 + 1]
                den[a_idx] += T - own_s
            else:
                ca = slab[a_idx]
                for ci in range(C):
                    m = ca != ci
                    den[a_idx[m]] += oo[m, OC_SEG + t * 3 + ci]
        asl = slice(k * APC, (k + 1) * APC)
        for t in range(NOWN):
            a_idx = k * APC + t * 128 + tidx            # own anchors
            den[a_idx] += oo[:, OC_BBT + t] - oo[:, OC_SELFE + t]
            sdiag_own[a_idx] = oo[:, OC_SDIAG + t]
            cls = slab[a_idx]
            raw3_own[a_idx] = oo[tidx, OC_RAW3 + t * 3 + cls]

    # dummy correction: the sdum zero columns sit in the class-2 segment
    # (exp(-inv_t) each, per core); anchors of class 2 already exclude it
    if sdum > 0:
        m2 = slab != 2
        den[m2] -= NCORES * sdum * np.exp(-inv64[m2])

    # leftover (overflow) bank columns, folded in exactly on the host
    ov_cols, ov_cls = [], []
    for c in range(C):
        lo, hi = cls_off[c] + NCORES * q[c], cls_off[c] + mcnt[c]
        for j in range(lo, hi):
            ov_cols.append(j)
            ov_cls.append(c)
    if ov_cols:
        bq = bankq_f[ov_cols]                           # [n_ov, D]
        s_ov = embq_f @ bq.T                            # [B, n_ov]
        terms = np.exp(inv64[:, None] * (s_ov.astype(np.float64) - 1.0))
        mask = slab[:, None] != np.asarray(ov_cls)[None, :]
        den += (terms * mask).sum(axis=1)

    pos_cnt = (cnt[slab] - 1).astype(np.float64)
    pos_sum = raw3_own - sdiag_own
    pos_mean = pos_sum / np.maximum(pos_cnt, 1.0)
    log_denom = inv64 + np.log(den)
    coef = BASE_TEMP * inv64
    loss_i = coef * (log_denom - pos_mean)
    valid = pos_cnt > 0
    n_valid = int(valid.sum())
    loss = (loss_i * valid).sum() / max(n_valid, 1)
    return np.float32(loss)


# revision 21
# speedup vs baseline: 1.1836x; 1.1836x over previous
"""ClassBalancedSupConLoss on 8 TRN2 NeuronCores (Bass/Tile), v3.

Sharding: the BANK is column-sharded across the 8 cores (2048 cols each,
class-balanced with uniform cut positions), every core holds the full
(class-sorted) batch as matmul columns plus its own 256 anchors as
stationary weights. Each core computes, for ALL 2048 anchors, exp-sums
against its bank slice, and for its OWN anchors the batch (bb) exp-sum
total, self term, and positives row-sums. The host (numpy, fp64)
assembles denominators, logs, and the masked mean.

Work split per [128, 2048] chunk (18 per core):
  PE : 4 x [128, 512] matmuls into a rotating PSUM tile
  ACT: one Exp pass with accum_out = chunk total (free during ACTIVATE)
  DVE: own-class-range segment reduce(s) of the bf16 exp output
Pure anchor tiles (all 128 sorted anchors share a class -- 14 of 16)
need only ONE fixed-range DVE reduce: denominator = total - own-class.
The <= 2 class-straddling tiles fall back to 3 per-segment reduces.
bb chunks need NO reduce at all: denominator_bb = total - selfe, with
selfe = exp(inv_t*(s_ii-1)) computed from the same rounded operands so
the ~1.0 self term cancels at ACT-internal precision.

Class balancing: per class c every core gets exactly q_c =
floor(mcnt_c/8) bank columns; the per-core shortfall (2048 - sum q_c)
is zero-vector dummy columns whose exact exp(-inv_t) contribution the
host subtracts, and the <= 21 leftover real columns are folded in on
the host. Cut positions q0, q0+q1 are therefore compile-time constants
shared by all cores (SPMD-safe fixed-range reduces).

Numerics: matmul inputs fp8 e4m3 (logit noise washes out over the
18k-term sums; positives/self are consistent because host corrections
reuse the same quantized operands); exp outputs bf16; device sums fp32;
host assembly fp64.
"""

import os
import numpy as np

import concourse.bass as bass  # noqa: F401
from concourse import bacc
import concourse.mybir as mybir
import concourse.tile as tile
from concourse.bass_utils import run_bass_kernel_spmd

B, D, M, C = 2048, 128, 16384, 3
NCORES = 8
APC = B // NCORES          # own anchors per core = 256
NT = B // 128              # anchor tiles = 16 (all anchors)
NOWN = APC // 128          # own anchor tiles = 2
CH = 512                   # matmul free chunk (one PSUM bank)
W = 2048                   # chunk width = one PSUM [128, 2048] tile
BASE_TEMP = 0.07

F32 = mybir.dt.float32
BF16 = mybir.dt.bfloat16
AF = mybir.ActivationFunctionType
ALU = mybir.AluOpType
AX = mybir.AxisListType

# "f8"  : fp8 e4m3 matmul inputs (default)
# "bf16": bfloat16 matmul inputs (2x DMA bytes, less logit noise)
MM_MODE = os.environ.get("SUPCON_MM_MODE", "f8")
WARMUP = int(os.environ.get("SUPCON_WARMUP", "0"))
# number of pure bank chunks whose exp runs on the DVE (Schraudolph
# int16/bf16 bit-trick) instead of the saturated ACT engine
NOFF = int(os.environ.get("SUPCON_OFFLOAD", "0"))

# bf16-domain Schraudolph exp: bitcast(int16(A16*y + B16)) ~= e^y
A16 = 128.0 / np.log(2.0)
B16_BASE = 127.0 * 128.0

LAST_EXEC_TIME_NS = None   # set by kernel() when SUPCON_TRACE=1

# oout column layout (per core, [128, OC] fp32)
OC_SEG = 0                 # 16 tiles x 3: pure = (T, own, -) / straddle = (s0, s1, s2)
OC_BBT = 48                # 2 own tiles: bb totals
OC_SELFE = 50              # 2
OC_RAW3 = 52               # 2 own tiles x 3 = 6
OC_SDIAG = 58              # 2
OC = 60


def _install_trace_shim():
    """Register the NTFF profile hook that this image's antenv lacks."""
    import sys
    import types
    import ctypes
    import contextlib

    try:
        from antenv.axon_hooks import get_axon_ntff_profile_hook  # noqa: F401
        return True  # real module exists
    except ImportError:
        pass

    so_path = "/opt/axon/libaxon_pjrt.so"
    if not os.path.exists(so_path):
        return False
    lib = ctypes.CDLL(so_path)
    if not hasattr(lib, "axon_start_nrt_profile"):
        return False
    lib.axon_start_nrt_profile.argtypes = [
        ctypes.POINTER(ctypes.c_int64),
        ctypes.c_size_t,
    ]
    lib.axon_start_nrt_profile.restype = ctypes.c_int64
    lib.axon_stop_nrt_profile.argtypes = [ctypes.c_char_p]
    lib.axon_stop_nrt_profile.restype = ctypes.c_int64

    @contextlib.contextmanager
    def _hook(output_dir, device_ids):
        import jax

        jax.devices()
        if device_ids:
            ids = (ctypes.c_int64 * len(device_ids))(*device_ids)
            rc = lib.axon_start_nrt_profile(ids, len(device_ids))
        else:
            rc = lib.axon_start_nrt_profile(None, 0)
        if rc != 0:
            raise RuntimeError(f"axon_start_nrt_profile rc={rc}")
        try:
            yield
        finally:
            n = lib.axon_stop_nrt_profile(str(output_dir).encode())
            print(f"profile: {n} file(s) written to {output_dir}", file=sys.stderr)

    _state = {"hook": _hook}
    mod = types.ModuleType("antenv.axon_hooks")
    mod.get_axon_ntff_profile_hook = lambda: _state["hook"]
    mod.set_axon_ntff_profile_hook = lambda h: _state.update(hook=h)
    sys.modules["antenv.axon_hooks"] = mod
    import antenv

    antenv.axon_hooks = mod

    import concourse.bass_utils as bu

    bu.upload_artifacts = lambda tmpdir: tmpdir
    return True


def _build(c1, c2, tile_cls, off_tiles, mm_mode):
    """c1/c2: class cuts in every core's bank slice; tile_cls[t]: class of
    anchor tile t if pure, else None (straddles a batch class boundary);
    off_tiles: pure bank tiles whose exp runs on the DVE (bit-trick)."""
    import ml_dtypes  # noqa: F401

    in_dt = mybir.dt.float8e4 if mm_mode == "f8" else BF16
    I16 = mybir.dt.int16

    AW = APC + 8               # anchT width: 256 own + 3 gT + 5 pad
    # invt | ninvt | invt_own | ninvt_own | sA | sB | eye
    NV = 4 * NT + 2 * NOWN + 128

    nc = bacc.Bacc()
    embT_d = nc.declare_dram_parameter("embT", [D, W], in_dt, isOutput=False)
    anchT_d = nc.declare_dram_parameter("anchT", [D, AW], in_dt, isOutput=False)
    bankT_d = nc.declare_dram_parameter("bankT", [D, W], in_dt, isOutput=False)
    vecs_d = nc.declare_dram_parameter("vecs", [128, NV], F32, isOutput=False)
    oout_d = nc.declare_dram_parameter("oout", [128, OC], F32, isOutput=True)

    seg_r = [(0, c1), (c1, c2), (c2, W)]

    with tile.TileContext(nc) as tc:
        with (
            tc.tile_pool(name="big", bufs=1) as bigp,
            tc.tile_pool(name="sm", bufs=1) as smp,
            tc.tile_pool(name="scr", bufs=2) as scrp,
            tc.tile_pool(name="i16", bufs=2) as i16p,
            tc.tile_pool(name="ps", bufs=2, space="PSUM") as psp,
        ):
            emb_t = bigp.tile([D, W], in_dt, tag="embT")
            anch_t = bigp.tile([D, AW], in_dt, tag="anchT")
            bank_t = bigp.tile([D, W], in_dt, tag="bankT")
            vecs_t = smp.tile([128, NV], F32, tag="vecs")
            o = [0]

            def vslice(w):
                a = o[0]; o[0] += w
                return vecs_t[:, a:a + w]
            invt_t = vslice(NT)
            ninvt_t = vslice(NT)
            invo_t = vslice(NOWN)
            ninvo_t = vslice(NOWN)
            sA_t = vslice(NT)
            sB_t = vslice(NT)
            eye_t = vslice(128)
            # garbage-operand warmup tiles (never written)
            junkw_t = bigp.tile([128, 128], in_dt, tag="junkw")
            junkx_t = bigp.tile([128, CH], in_dt, tag="junkx")

            oout_t = smp.tile([128, OC], F32, tag="oout")
            eyemul = smp.tile([128, 128], F32, tag="eyemul")
            warm = smp.tile([128, 1], F32, tag="warm")
            scr2 = smp.tile([128, W], BF16, tag="scr2")
            sdiag = [smp.tile([128, 1], F32, tag=f"sdiag{t}", name=f"sdiag{t}")
                     for t in range(NOWN)]

            # ACT first: exp table load + warm activation on junk data, so
            # the ~2.7us table load runs during the input DMA window.
            nc.vector.memset(junkx_t[:, 0:1], 0.0)
            nc.scalar.activation(warm[:], junkx_t[:, 0:1], AF.Exp)

            # input DMA: sync queue carries vecs + emb + anchT (needed
            # first), scalar queue carries the bank slice (needed later).
            H = W // 2
            nc.sync.dma_start(out=vecs_t[:], in_=vecs_d[:])
            nc.sync.dma_start(out=anch_t[:], in_=anchT_d[:])
            nc.sync.dma_start(out=emb_t[:, 0:H], in_=embT_d[:, 0:H])
            nc.sync.dma_start(out=emb_t[:, H:W], in_=embT_d[:, H:W])
            nc.scalar.dma_start(out=bank_t[:], in_=bankT_d[:])

            # PE warmup on garbage operands (HAM clock-gate opener)
            if WARMUP:
                nc.vector.memset(junkw_t[:], 0.0)
                nc.vector.memset(junkx_t[:], 0.0)
                warm_ps = psp.tile([128, W], F32, tag="chunk", name="warm_ps")
                for w in range(WARMUP):
                    nc.tensor.matmul(
                        warm_ps[:, (w % 4) * CH:((w % 4) + 1) * CH],
                        junkw_t[:], junkx_t[:], start=True, stop=True,
                    )

            def ts_sum(src, width, col):
                """oout[col] = sum(src) on the DVE (TENSOR_REDUCE, 1x)."""
                nc.vector.reduce_sum(oout_t[:, col:col + 1], src, axis=AX.X)

            def emit_chunk(lhs, moving, sc, bi, accum, reduces, off=None):
                """[128, 2048] chunk: 4 matmuls + exp + range sums.

                ACT path: Exp (scale sc / bias bi) with optional accum col,
                then DVE range sums of the bf16 output.
                DVE path (off=(sa, sb)): Schraudolph int16 convert + bitcast
                bf16 range sums; ACT untouched."""
                ps = psp.tile([128, W], F32, tag="chunk", name="ps")
                for q in range(W // CH):
                    nc.tensor.matmul(
                        ps[:, q * CH:(q + 1) * CH], lhs,
                        moving[:, q * CH:(q + 1) * CH],
                        start=True, stop=True,
                    )
                if off is not None:
                    sa, sb = off
                    i16 = i16p.tile([128, W], I16, tag="i16", name="i16")
                    nc.vector.tensor_scalar(
                        out=i16[:], in0=ps[:], scalar1=sa, scalar2=sb,
                        op0=ALU.mult, op1=ALU.add)
                    for (a, b, col) in reduces:
                        ts_sum(i16[:, a:b].bitcast(BF16), b - a, col)
                else:
                    scr = scrp.tile([128, W], BF16, tag="scr", name="scr")
                    kw = {}
                    if accum is not None:
                        kw["accum_out"] = oout_t[:, accum:accum + 1]
                    nc.scalar.activation(
                        scr[:], ps[:], AF.Exp, bias=bi, scale=sc, **kw)
                    for (a, b, col) in reduces:
                        ts_sum(scr[:, a:b], b - a, col)

            def own(t):
                return anch_t[:, t * 128:(t + 1) * 128]

            # bb chunks for the 2 own tiles: total only (host does T - selfe)
            for t in range(NOWN):
                emit_chunk(own(t), emb_t,
                           invo_t[:, t:t + 1], ninvo_t[:, t:t + 1],
                           OC_BBT + t, [])

            # bank chunks for all 16 anchor tiles.  Pure tiles: ACT accum
            # gives the chunk total, one DVE reduce gives the own-class
            # segment (host: den += T - own).
            for t in range(NT):
                lhs = emb_t[:, t * 128:(t + 1) * 128]
                sc, bi = invt_t[:, t:t + 1], ninvt_t[:, t:t + 1]
                if tile_cls[t] is not None:
                    a, bnd = seg_r[tile_cls[t]]
                    emit_chunk(lhs, bank_t, sc, bi, OC_SEG + t * 3,
                               [(a, bnd, OC_SEG + t * 3 + 1)])
                else:
                    rd = [(a, bnd, OC_SEG + t * 3 + ci)
                          for ci, (a, bnd) in enumerate(seg_r) if bnd > a]
                    emit_chunk(lhs, bank_t, sc, bi, None, rd)

            # epilogue: self-similarity diag + positives row-sums
            post_ps = psp.tile([128, W], F32, tag="chunk", name="post_ps")
            for t in range(NOWN):
                nc.tensor.matmul(
                    post_ps[:, t * 128:(t + 1) * 128], own(t), own(t),
                    start=True, stop=True,
                )
            for t in range(NOWN):
                nc.tensor.matmul(
                    post_ps[:, 256 + t * C:256 + (t + 1) * C], own(t),
                    anch_t[:, APC:APC + C], start=True, stop=True,
                )
            for t in range(NOWN):
                nc.vector.tensor_mul(
                    eyemul[:], post_ps[:, t * 128:(t + 1) * 128], eye_t[:])
                nc.vector.reduce_sum(sdiag[t][:], eyemul[:], axis=AX.X)
                nc.scalar.activation(
                    oout_t[:, OC_SELFE + t:OC_SELFE + t + 1], sdiag[t][:],
                    AF.Exp, bias=ninvo_t[:, t:t + 1], scale=invo_t[:, t:t + 1],
                )
                nc.vector.tensor_copy(
                    out=oout_t[:, OC_SDIAG + t:OC_SDIAG + t + 1], in_=sdiag[t][:])
            nc.vector.tensor_copy(
                out=oout_t[:, OC_RAW3:OC_RAW3 + NOWN * C],
                in_=post_ps[:, 256:256 + NOWN * C])

            nc.sync.dma_start(out=oout_d[:], in_=oout_t[:])

    nc.compile()
    return nc


def kernel(embeddings, labels, bank_embs, bank_labels, class_temps):
    global LAST_EXEC_TIME_NS
    import ml_dtypes

    f8 = ml_dtypes.float8_e4m3
    in_np = f8 if MM_MODE == "f8" else ml_dtypes.bfloat16

    emb = np.asarray(embeddings, dtype=np.float32)
    bank = np.asarray(bank_embs, dtype=np.float32)
    lab = np.asarray(labels).astype(np.int64).ravel()
    blab = np.asarray(bank_labels).astype(np.int64).ravel()
    ct = np.asarray(class_temps, dtype=np.float32).ravel()

    # sort batch and bank by class
    bord = np.argsort(lab, kind="stable")
    slab = lab[bord]
    emb_s = emb[bord]                                  # [B, D] f32, sorted
    cnt = np.bincount(lab, minlength=C)
    mord = np.argsort(blab, kind="stable")
    bank_s = bank[mord]
    mcnt = np.bincount(blab, minlength=C)

    # per-core class quotas (even, for 4B-aligned bf16 reduce ranges)
    q = ((mcnt // NCORES) // 2 * 2).astype(np.int64)   # [3]
    sdum = int(W - q.sum())                            # zero-dummy cols/core
    assert sdum >= 0
    c1, c2 = int(q[0]), int(q[0] + q[1])
    cls_off = np.concatenate([[0], np.cumsum(mcnt)[:-1]])

    # anchor-tile purity (compile-time, same for all cores)
    tile_cls = []
    for t in range(NT):
        c_lo, c_hi = slab[t * 128], slab[t * 128 + 127]
        tile_cls.append(int(c_lo) if c_lo == c_hi else None)

    # quantized operands (shared by device and host-side corrections)
    embq = emb_s.astype(in_np)                         # [B, D]
    bankq = bank_s.astype(in_np)
    embq_f = embq.astype(np.float32)
    bankq_f = bankq.astype(np.float32)
    g = np.stack([emb_s[slab == c].sum(axis=0) for c in range(C)], axis=1)
    gq = g.astype(in_np)                               # [D, 3]

    inv_t_all = (1.0 / ct[slab]).astype(np.float32)    # [B] per sorted anchor

    # DVE-offloaded pure bank tiles, spread across the chunk sequence
    pure = [t for t in range(NT) if tile_cls[t] is not None]
    noff = min(NOFF, len(pure))
    off_tiles = (set(pure[int(i)] for i in
                     np.linspace(0, len(pure) - 1, noff).round())
                 if noff > 0 else set())

    # per-class Schraudolph bias tuning: pick corr_c that zeroes the mean
    # relative error of bitcast(int16(A16*y + B16 - corr)) over the y
    # distribution of this class's logits (s ~ N(0, 1/sqrt(D)))
    corr_cls = np.zeros(C)
    if off_tiles:
        sgrid = np.linspace(-4.0, 4.0, 4001) / np.sqrt(D)
        wpdf = np.exp(-0.5 * (sgrid * np.sqrt(D)) ** 2)
        for c in range(C):
            it = 1.0 / float(ct[c])
            y = it * (sgrid - 1.0)
            exact = np.exp(y)
            wexp = wpdf * exact
            best, bestv = 0.0, np.inf
            for corr in np.linspace(0.0, 12.0, 121):
                i16v = np.clip(np.rint(A16 * y + B16_BASE - corr), 0, 32767)
                approx = i16v.astype(np.int16).view(ml_dtypes.bfloat16).astype(np.float64)
                bias = abs(np.sum(wpdf * approx) / np.sum(wpdf * exact) - 1.0)
                if bias < bestv:
                    best, bestv = corr, bias
            corr_cls[c] = best

    nc = _build(c1, c2, tile_cls, off_tiles, MM_MODE)

    eye128 = np.eye(128, dtype=np.float32)
    embT = np.ascontiguousarray(embq.T)                # [D, B], shared
    invt_cols = np.ascontiguousarray(inv_t_all.reshape(NT, 128).T)
    sA_cols = (A16 * invt_cols).astype(np.float32)
    corr_all = corr_cls[slab]
    sB_all = (B16_BASE - corr_all - A16 * inv_t_all.astype(np.float64))
    sB_cols = np.ascontiguousarray(sB_all.reshape(NT, 128).T).astype(np.float32)
    in_maps = []
    for k in range(NCORES):
        asl = slice(k * APC, (k + 1) * APC)
        anchT = np.zeros((D, APC + 8), dtype=in_np)
        anchT[:, 0:APC] = embq[asl].T
        anchT[:, APC:APC + C] = gq
        bankT = np.zeros((D, W), dtype=in_np)
        pos = 0
        for c in range(C):
            sel = bankq[cls_off[c] + k * q[c]: cls_off[c] + (k + 1) * q[c]]
            bankT[:, pos:pos + q[c]] = sel.T
            pos += int(q[c])
        ivo = inv_t_all[asl]
        vecs = np.concatenate([
            invt_cols, -invt_cols,
            np.ascontiguousarray(ivo.reshape(NOWN, 128).T),
            np.ascontiguousarray((-ivo).reshape(NOWN, 128).T),
            sA_cols, sB_cols,
            eye128,
        ], axis=1).astype(np.float32)
        in_maps.append({
            "embT": embT,
            "anchT": np.ascontiguousarray(anchT),
            "bankT": np.ascontiguousarray(bankT),
            "vecs": np.ascontiguousarray(vecs),
        })

    trace = os.environ.get("SUPCON_TRACE", "0") == "1"
    if trace:
        trace = _install_trace_shim()
    res = run_bass_kernel_spmd(nc, in_maps, core_ids=list(range(NCORES)), trace=trace)
    LAST_EXEC_TIME_NS = res.exec_time_ns

    # ---- host assembly (fp64) ----
    inv64 = inv_t_all.astype(np.float64)
    den = np.zeros(B, dtype=np.float64)
    raw3_own = np.zeros(B, dtype=np.float64)
    sdiag_own = np.zeros(B, dtype=np.float64)
    tidx = np.arange(128)
    for k in range(NCORES):
        oo = np.asarray(res.results[k]["oout"], dtype=np.float64)  # [128, OC]
        for t in range(NT):
            a_idx = t * 128 + tidx
            if tile_cls[t] is not None:
                T = oo[:, OC_SEG + t * 3]
                own_s = oo[:, OC_SEG + t * 3 + 1]
                den[a_idx] += T - own_s
            else:
                ca = slab[a_idx]
                for ci in range(C):
                    m = ca != ci
                    den[a_idx[m]] += oo[m, OC_SEG + t * 3 + ci]
        asl = slice(k * APC, (k + 1) * APC)
        for t in range(NOWN):
            a_idx = k * APC + t * 128 + tidx            # own anchors
            den[a_idx] += oo[:, OC_BBT + t] - oo[:, OC_SELFE + t]
            sdiag_own[a_idx] = oo[:, OC_SDIAG + t]
            cls = slab[a_idx]
            raw3_own[a_idx] = oo[tidx, OC_RAW3 + t * 3 + cls]

    # dummy correction: the sdum zero columns sit in the class-2 segment
    # (exp(-inv_t) each, per core); anchors of class 2 already exclude it
    if sdum > 0:
        m2 = slab != 2
        den[m2] -= NCORES * sdum * np.exp(-inv64[m2])

    # leftover (overflow) bank columns, folded in exactly on the host
    ov_cols, ov_cls = [], []
    for c in range(C):
        lo, hi = cls_off[c] + NCORES * q[c], cls_off[c] + mcnt[c]
        for j in range(lo, hi):
            ov_cols.append(j)
            ov_cls.append(c)
    if ov_cols:
        bq = bankq_f[ov_cols]                           # [n_ov, D]
        s_ov = embq_f @ bq.T                            # [B, n_ov]
        terms = np.exp(inv64[:, None] * (s_ov.astype(np.float64) - 1.0))
        mask = slab[:, None] != np.asarray(ov_cls)[None, :]
        den += (terms * mask).sum(axis=1)

    pos_cnt = (cnt[slab] - 1).astype(np.float64)
    pos_sum = raw3_own - sdiag_own
    pos_mean = pos_sum / np.maximum(pos_cnt, 1.0)
    log_denom = inv64 + np.log(den)
    coef = BASE_TEMP * inv64
    loss_i = coef * (log_denom - pos_mean)
    valid = pos_cnt > 0
    n_valid = int(valid.sum())
    loss = (loss_i * valid).sum() / max(n_valid, 1)
    return np.float32(loss)
